# revision 50
# baseline (speedup 1.0000x reference)
"""Trainium2 Bass kernel for a post-norm transformer encoder layer with RoPE.

B=2, S=2048, D=1024, H=16, Dh=64, F=4096, fp32 in/out.

Sharding (8 cores, no collectives): core c handles batch b=c//4 and query block
qb=c%4 (512 queries). Each core recomputes K/V for its full batch, computes
Q/attention/out_proj/LN/FFN for its own 512 rows.

v2 design (vs baseline):
  - x loaded ONCE per rep in bf16 (packed [sc][kt] chunk layout, 4x 1MB DMAs
    on the ACT HWDGE ring) + one f32 copy of the qb chunk for the residual;
    Q-projection runs first from the qb chunk; K and V projections read the
    same resident x tiles.  QKV + out_proj matmuls in bf16; FFN stays f32r.
  - K^T / Q^T / V_aug / probs / rope tables / ATT in BF16 (halves SBUF and
    doubles DVE throughput).
  - RoPE 32-row swaps via gpsimd (SWDGE) SBUF->SBUF DMAs - off the HWDGE ring.
  - Attention: ACT engine does ONLY the exp (128x [128,1024] psum->bf16 ops).
    Softmax denominator reciprocal on DVE (reads PSUM), normalize via DVE
    with crossed base partitions (PSUM in0 permits it), per-head.
  - out_proj (E) interleaved into attention: po psum accumulates per ot as
    head-pair ATT tiles complete; bias via DVE tensor_scalar_add; LN stats
    Square on DVE; rstd = Exp(-0.5*Ln(var+eps)) so the whole kernel uses one
    ACT table set (natural_log_exp_and_others) - no table reloads.
  - Weights host-packed: wq/wk/wv/ow are single 2MB bf16 DMAs, w1/w2 stream
    as 8x2MB f32 blocks; ~40 DMAs/rep vs 239 in the baseline (HWDGE
    descriptor prep at ~0.63us each was serializing phase transitions).
"""
import sys, os
for _p in ('/opt/trn_rl_repo', '/root/.axon_site/_ro/trn_rl_repo'):
    if os.path.isdir(_p) and _p not in sys.path:
        sys.path.insert(0, _p)

import numpy as np
import ml_dtypes
from contextlib import ExitStack

import concourse.bacc as bacc
import concourse.mybir as mybir
import concourse.tile as tile
from concourse.bass_utils import run_bass_kernel_spmd

# Pin every activation this kernel uses to the single table set that covers
# them all (natural_log_exp_and_others: exp+ln+identity+relu+square), by
# stripping those functions from every other set before the table-load pass
# runs. Otherwise the chooser alternates exp_and_others <-> natural_log and
# inserts ~4 LoadActFuncSet (~2.7us each) per rep around the LayerNorms.
import functools as _ft
import concourse.hw_specs as _hw

_PIN_SET = "natural_log_exp_and_others"


@_ft.cache
def _pinned_act_tables(arch):
    tabs = {k: set(v) for k, v in _hw.get_activation_tables(arch).items()}
    if _PIN_SET in tabs:
        _AF = mybir.ActivationFunctionType
        ours = {_AF.Exp, _AF.Ln, _AF.Identity, _AF.Relu, _AF.Square}
        if ours <= tabs[_PIN_SET]:
            for name, funcs in tabs.items():
                if name != _PIN_SET:
                    funcs -= ours
    return tabs


bacc.get_activation_tables = _pinned_act_tables

F32 = mybir.dt.float32
F32R = mybir.dt.float32r
BF16 = mybir.dt.bfloat16
AF = mybir.ActivationFunctionType
ALU = mybir.AluOpType

B, S, D, H, Dh, F = 2, 2048, 1024, 16, 64, 4096
Q = 512                 # queries per core
NT_D = D // 128         # 8 d-tiles
NT_S = S // 128         # 16 s-tiles
NC_S = S // 512         # 4 s-chunks
NT_F = F // 128         # 32 f-tiles
LN_EPS = 1e-5
ROPE_BASE = 10000.0

# bpack column layout (each vector of length 1024 -> 8 cols, one per d-tile)
_BP = {"bq": 0, "bk": 8, "ob": 16, "b2": 24, "g1": 32, "be1": 40,
       "g2": 48, "be2": 56, "b1": 64, "ones": 96}
BP_COLS = 112

_CACHE = {}


def _build(repeat=1):
    nc = bacc.Bacc("TRN2", target_bir_lowering=False, debug=False, num_devices=8)

    def inp(name, shape, dt):
        return nc.dram_tensor(name, list(shape), dt, kind="ExternalInput")

    # packed x (bf16): [p, sc*4096 + kt*512 + j]; chunk 0 is this core's
    # q-block (host permutes chunks; K rope tables are permuted to match)
    xPb = inp("xPb", (128, NC_S * 4096), BF16)
    xQf = inp("xQf", (128, 4096), F32R)      # qb chunk f32 (residual)
    wqP = inp("wqP", (128, D * NT_D), BF16)  # [p, kt*1024 + c] = wT[kt*128+p, c]
    wkP = inp("wkP", (128, D * NT_D), BF16)
    wvP = inp("wvP", (128, D * NT_D), BF16)
    owP = inp("owP", (128, D * NT_D), BF16)
    w1P = inp("w1P", (128, NT_D * F), BF16)  # w1T packed: [p, kt*F + c]
    w2P = inp("w2P", (128, NT_F * D), BF16)  # w2T packed: [p, ft*D + c]
    bpack = inp("bpack", (128, BP_COLS), F32R)
    cosKb = inp("cosKb", (128, S), BF16)     # [cos;cos;cos;cos] blocks of 32
    sinDKb = inp("sinDKb", (128, S), BF16)   # [-sin;+sin;-sin;+sin]
    yT = nc.dram_tensor("yT", [D, Q], F32, kind="ExternalOutput")

    with tile.TileContext(nc) as tc, ExitStack() as octx:
        pconst = octx.enter_context(tc.tile_pool(name="pconst", bufs=1))

        bp = pconst.tile([128, BP_COLS], F32R, tag="bp")
        nc.sync.dma_start(bp[:], bpack.ap())

        def bcol(key, i):          # (128,1) f32 bias view
            c = _BP[key] + i
            return bp[:, c:c + 1].bitcast(F32)

        ones_c = bp[:, _BP["ones"]:_BP["ones"] + 1]   # f32r stationary
        ones16 = bp[:, _BP["ones"]:_BP["ones"] + 16]  # f32r ones cols

        for _rep in range(repeat):
            # Right-stack pools, opened in reverse-close order (LIFO):
            #   s_EF (wo/H1HR)   closes mid-F        -> bottom
            #   s_big (KT/VA/..) closes after D+E    -> middle
            #   s_x  (x chunks)  closes after V proj -> top
            s_EF = ExitStack()      # LEFT-stack pools opened just before D
            s_big = ExitStack()
            s_x = ExitStack()

            pXQ = s_big.enter_context(tc.tile_pool(name="pXQ", bufs=1,
                                                   side="right"))
            ptbl = s_big.enter_context(tc.tile_pool(name="ptbl", bufs=1,
                                                    side="right"))
            pQT = s_big.enter_context(tc.tile_pool(name="pQT", bufs=1,
                                                   side="right"))
            pKT = s_big.enter_context(tc.tile_pool(name="pKT", bufs=1,
                                                   side="right"))
            pVA = s_big.enter_context(tc.tile_pool(name="pVA", bufs=1,
                                                   side="right"))
            pATT = s_big.enter_context(tc.tile_pool(name="pATT", bufs=1,
                                                    side="right"))
            px = s_x.enter_context(tc.tile_pool(name="px", bufs=1,
                                                side="right"))

            # ---- prologue DMAs (program order sets HWDGE ring order) ----
            # ACT ring: x chunks + rope tables. SP ring: weights in
            # consumption order (wq, wk, wv, ow, w1, w2).
            x_t = []
            for sc in range(NC_S):
                t = px.tile([128, 4096], BF16, tag=f"x{sc}", name=f"xc{sc}")
                nc.scalar.dma_start(t[:], xPb.ap()[:, sc * 4096:(sc + 1) * 4096])
                x_t.append(t)
            cosk_t = ptbl.tile([128, S], BF16, tag="cosk")
            nc.scalar.dma_start(cosk_t[:], cosKb.ap())
            sink_t = ptbl.tile([128, S], BF16, tag="sink")
            nc.scalar.dma_start(sink_t[:], sinDKb.ap())
            # block 0 of the permuted K tables IS this core's q-block
            cosq_t, sinq_t = cosk_t[:, 0:Q], sink_t[:, 0:Q]

            # wq/wk ride the ACT ring too: their preps queue right behind the
            # x chunks, so the NEXT rep's copies prefetch during this rep's
            # D/F instead of waiting behind w1/w2 preps on the SP ring.
            # xq (residual) loads AFTER them: its slot WAR clears late (end of
            # E) and would otherwise head-of-line block the weight preps.
            pwq = ExitStack()
            wqp = pwq.enter_context(tc.tile_pool(name="wq", bufs=1))
            wq_t = wqp.tile([128, 8192], BF16, tag="wq", name="wqt")
            nc.scalar.dma_start(wq_t[:], wqP.ap())

            QT = [pQT.tile([128, Q], BF16, tag=f"qt{i}", name=f"QT{i}")
                  for i in range(NT_D)]
            KT = [pKT.tile([128, S], BF16, tag=f"kt{i}", name=f"KT{i}")
                  for i in range(NT_D)]
            VA = [pVA.tile([128, 16 * 65], BF16, tag=f"va{i}", name=f"VA{i}")
                  for i in range(NT_S)]

            # ============ Phase C: Q^T proj + rope (qb chunk) ============
            with ExitStack() as ctx:
                scr = ctx.enter_context(tc.tile_pool(name="scrC", bufs=1))
                psC = ctx.enter_context(tc.tile_pool(name="psC", bufs=1,
                                                     space="PSUM"))
                for dt in range(NT_D):
                    pq = psC.tile([128, Q], F32, tag=f"pq{dt % 2}",
                                  name=f"pqC{dt}")
                    for kt in range(NT_D):
                        nc.tensor.matmul(
                            pq[:],
                            lhsT=wq_t[:, kt * 1024 + dt * 128:
                                      kt * 1024 + (dt + 1) * 128],
                            rhs=x_t[0][:, kt * 512:(kt + 1) * 512],
                            start=(kt == 0), stop=(kt == NT_D - 1))
                    raw = scr.tile([128, Q], BF16, tag="rawq", bufs=3,
                                   name=f"rwC{dt}")
                    nc.scalar.activation(raw[:], pq[:], AF.Identity,
                                         bias=bcol("bq", dt))
                    sw = scr.tile([128, Q], BF16, tag="swq", bufs=3,
                                  name=f"swC{dt}")
                    for a, bb in ((0, 32), (64, 96)):
                        nc.gpsimd.dma_start(sw[a:a + 32, :], raw[bb:bb + 32, :])
                        nc.gpsimd.dma_start(sw[bb:bb + 32, :], raw[a:a + 32, :])
                    nc.vector.tensor_mul(raw[:], raw[:], cosq_t)
                    nc.vector.tensor_mul(sw[:], sw[:], sinq_t)
                    nc.vector.tensor_add(QT[dt][:], raw[:], sw[:])
            pwq.close()

            pwk = ExitStack()
            wkp = pwk.enter_context(tc.tile_pool(name="wk", bufs=1))
            wk_t = wkp.tile([128, 8192], BF16, tag="wk", name="wkt")
            nc.scalar.dma_start(wk_t[:], wkP.ap())
            xq_t = pXQ.tile([128, 4096], F32R, tag="xq", name="xqchunk")
            nc.scalar.dma_start(xq_t[:], xQf.ap())

            # ============ Phase A: K^T proj + rope (full S) ============
            with ExitStack() as ctx:
                scr = ctx.enter_context(tc.tile_pool(name="scrA", bufs=1))
                psA = ctx.enter_context(tc.tile_pool(name="psA", bufs=1,
                                                     space="PSUM"))
                for dt in range(NT_D):
                    pk = psA.tile([128, S], F32, tag=f"pk{dt % 2}",
                                  name=f"pkA{dt}")
                    for kt in range(NT_D):
                        for sc in range(NC_S):
                            nc.tensor.matmul(
                                pk[:, sc * 512:(sc + 1) * 512],
                                lhsT=wk_t[:, kt * 1024 + dt * 128:
                                          kt * 1024 + (dt + 1) * 128],
                                rhs=x_t[sc][:, kt * 512:(kt + 1) * 512],
                                start=(kt == 0), stop=(kt == NT_D - 1))
                    raw = scr.tile([128, S], BF16, tag="rawk", bufs=3,
                                   name=f"rwA{dt}")
                    nc.scalar.activation(raw[:], pk[:], AF.Identity,
                                         bias=bcol("bk", dt))
                    sw = scr.tile([128, S], BF16, tag="swk", bufs=3,
                                  name=f"swA{dt}")
                    for a, bb in ((0, 32), (64, 96)):
                        nc.gpsimd.dma_start(sw[a:a + 32, :], raw[bb:bb + 32, :])
                        nc.gpsimd.dma_start(sw[bb:bb + 32, :], raw[a:a + 32, :])
                    nc.vector.tensor_mul(raw[:], raw[:], cosk_t[:])
                    nc.vector.tensor_mul(sw[:], sw[:], sink_t[:])
                    nc.vector.tensor_add(KT[dt][:], raw[:], sw[:])
            pwk.close()

            pwv = ExitStack()
            wvp = pwv.enter_context(tc.tile_pool(name="wv", bufs=1))
            wv_t = wvp.tile([128, 8192], BF16, tag="wv", name="wvt")
            nc.sync.dma_start(wv_t[:], wvP.ap())

            # ============ Phase B: V proj into V_aug ============
            with ExitStack() as ctx:
                psB = ctx.enter_context(tc.tile_pool(name="psB", bufs=1,
                                                     space="PSUM"))
                for sg in range(NC_S):
                    for sl in range(4):
                        st = sg * 4 + sl
                        va3 = VA[st].rearrange("p (h c) -> p h c", c=65)
                        nc.scalar.activation(
                            va3[:, :, 64:65],
                            ones16.rearrange("p (h c) -> p h c", c=1),
                            AF.Identity)
                        pv = psB.tile([128, 1024], F32, tag="pv", bufs=4,
                                      name=f"pv{st}")
                        for n in range(2):
                            # v bias is folded into the out_proj bias on the
                            # host (ob' = out_b + out_w @ bv): attn(v+b) =
                            # attn(v) + b exactly, since sum(probs) = 1.
                            for kt in range(NT_D):
                                nc.tensor.matmul(
                                    pv[:, n * 512:(n + 1) * 512],
                                    lhsT=x_t[sg][:, kt * 512 + sl * 128:
                                                 kt * 512 + (sl + 1) * 128],
                                    rhs=wv_t[:, kt * 1024 + n * 512:
                                             kt * 1024 + (n + 1) * 512],
                                    start=(kt == 0), stop=(kt == NT_D - 1))
                            nc.scalar.activation(
                                va3[:, n * 8:(n + 1) * 8, 0:64],
                                pv[:, n * 512:(n + 1) * 512]
                                .rearrange("p (h c) -> p h c", c=64),
                                AF.Identity)
            pwv.close()
            s_x.close()   # free x chunks

            # ow + HR/H1b live on the LEFT stack, opened only now (they're
            # dead weight during ABC); they close mid-F via s_EF, after the
            # F-phase pools (opened later) have closed -- valid LIFO.
            powo = s_EF.enter_context(tc.tile_pool(name="wo", bufs=1))
            hrp = s_EF.enter_context(tc.tile_pool(name="pHR", bufs=1))

            # SP ring: ow (needed in interleaved E), then w1/w2 prefetch
            ow_t = powo.tile([128, 8192], BF16, tag="ow", name="owt")
            nc.sync.dma_start(ow_t[:], owP.ap())

            # ======= Phase D: attention + interleaved out_proj (E) =======
            ATT = [pATT.tile([128, Q], BF16, tag=f"att{i}", name=f"ATT{i}")
                   for i in range(NT_D)]
            HR = [hrp.tile([128, Q], F32R, tag=f"hr{i}", name=f"HR{i}")
                  for i in range(NT_D)]
            H1 = HR   # LN1 normalizes HR in place (through scratch t1)
            with ExitStack() as ctx:
                ptp = ctx.enter_context(tc.tile_pool(name="ptp", bufs=1))
                nrm = ctx.enter_context(tc.tile_pool(name="nrm", bufs=1))
                scr = ctx.enter_context(tc.tile_pool(name="scrD", bufs=1))
                stat = ctx.enter_context(tc.tile_pool(name="statD", bufs=1))
                psS = ctx.enter_context(tc.tile_pool(name="psS", bufs=1,
                                                     space="PSUM"))
                psAt = ctx.enter_context(tc.tile_pool(name="psAt", bufs=1,
                                                      space="PSUM"))
                psE = ctx.enter_context(tc.tile_pool(name="psE", bufs=1,
                                                     space="PSUM"))
                for h in range(H):
                    dt, po = h // 2, (h % 2) * 64
                    pa = psAt.tile([65, 512], F32, tag="pa", bufs=3,
                                   name=f"pa{h}")
                    for kcp in range(NT_S // 2):
                        ps_t = psS.tile([128, 1024], F32, tag="ps", bufs=2,
                                        name=f"ps{h}_{kcp}")
                        for half in range(2):
                            kc = kcp * 2 + half
                            nc.tensor.matmul(
                                ps_t[:, half * 512:(half + 1) * 512],
                                lhsT=KT[dt][po:po + 64,
                                            kc * 128:(kc + 1) * 128],
                                rhs=QT[dt][po:po + 64, :],
                                start=True, stop=True)
                        pt_t = ptp.tile([128, 1024], BF16, tag="pt", bufs=5,
                                        name=f"pt{h}_{kcp}")
                        nc.scalar.activation(pt_t[:], ps_t[:], AF.Exp,
                                             scale=0.125)
                        for half in range(2):
                            kc = kcp * 2 + half
                            nc.tensor.matmul(
                                pa[:],
                                lhsT=VA[kc][:, h * 65:h * 65 + 65],
                                rhs=pt_t[:, half * 512:(half + 1) * 512],
                                start=(kc == 0), stop=(kc == NT_S - 1))
                    rec2 = nrm.tile([1, 512], F32, tag="rec2", bufs=2,
                                    name=f"rec2_{h}")
                    nc.vector.reciprocal(rec2[:], pa[64:65, :])
                    recb = nrm.tile([128, 512], F32, tag="recb", bufs=3,
                                    name=f"recb{h}")
                    nc.gpsimd.partition_broadcast(recb[:], rec2[:],
                                                  channels=128)
                    # psum in0 permits crossed partition bases on DVE
                    nc.vector.tensor_mul(ATT[dt][po:po + 64, :], pa[0:64, :],
                                         recb[po:po + 64, :])
                # out_proj: po_t[ot] accumulates over at in head order, so
                # Tile starts these matmuls as ATT tiles complete, filling PE
                # idle while ACT grinds the exps.
                for ot in range(NT_D):
                    po_t = psE.tile([128, Q], F32, tag="po", bufs=1,
                                    name=f"poE{ot}")
                    for at_ in range(NT_D):
                        nc.tensor.matmul(
                            po_t[:],
                            lhsT=ow_t[:, at_ * 1024 + ot * 128:
                                      at_ * 1024 + (ot + 1) * 128],
                            rhs=ATT[at_][:],
                            start=(at_ == 0), stop=(at_ == NT_D - 1))
                    ho = scr.tile([128, Q], F32, tag="ho", bufs=3,
                                  name=f"hoE{ot}")
                    nc.vector.tensor_scalar_add(ho[:], po_t[:], bcol("ob", ot))
                    nc.vector.tensor_add(HR[ot][:], ho[:],
                                         xq_t[:, ot * 512:(ot + 1) * 512]
                                         .bitcast(F32))
            # ---- LN1 stats + normalize (rstd via Ln+Exp: one table set) ----
            with ExitStack() as ctx:
                scr = ctx.enter_context(tc.tile_pool(name="scrE2", bufs=1))
                stat = ctx.enter_context(tc.tile_pool(name="statE2", bufs=1))
                psStat = ctx.enter_context(tc.tile_pool(name="psStatE", bufs=1,
                                                        space="PSUM"))
                pstat = psStat.tile([1, 1024], F32, tag="pstat")
                pSum, pSq = pstat[:, 0:512], pstat[:, 512:1024]
                for ot in range(NT_D):
                    sq = scr.tile([128, Q], F32R, tag="sq", bufs=2,
                                  name=f"sqE{ot}")
                    nc.vector.tensor_mul(sq[:], HR[ot][:].bitcast(F32),
                                         HR[ot][:].bitcast(F32))
                    nc.tensor.matmul(pSum, lhsT=ones_c, rhs=HR[ot][:],
                                     start=(ot == 0), stop=(ot == NT_D - 1))
                    nc.tensor.matmul(pSq, lhsT=ones_c, rhs=sq[:],
                                     start=(ot == 0), stop=(ot == NT_D - 1))
                mu = stat.tile([1, Q], F32, tag="mu")
                nc.vector.tensor_scalar_mul(mu[:], pSum, 1.0 / D)
                var = stat.tile([1, Q], F32, tag="var")
                nc.vector.tensor_scalar_mul(var[:], pSq, 1.0 / D)
                mu2 = stat.tile([1, Q], F32, tag="mu2")
                nc.vector.tensor_mul(mu2[:], mu[:], mu[:])
                nc.vector.tensor_sub(var[:], var[:], mu2[:])
                nc.vector.tensor_scalar_add(var[:], var[:], LN_EPS)
                lnv = stat.tile([1, Q], F32, tag="lnv")
                nc.scalar.activation(lnv[:], var[:], AF.Ln)
                rstd = stat.tile([1, Q], F32, tag="rstd")
                nc.scalar.activation(rstd[:], lnv[:], AF.Exp, scale=-0.5)
                muf = stat.tile([128, Q], F32, tag="muf")
                nc.gpsimd.partition_broadcast(muf[:], mu[:], channels=128)
                rstdf = stat.tile([128, Q], F32, tag="rstdf")
                nc.gpsimd.partition_broadcast(rstdf[:], rstd[:], channels=128)
                H1b = [hrp.tile([128, Q], BF16, tag=f"h1b{i}", name=f"H1b{i}")
                       for i in range(NT_D)]
                for ot in range(NT_D):
                    t1 = scr.tile([128, Q], F32, tag="t1", bufs=2,
                                  name=f"t1E{ot}")
                    nc.vector.tensor_sub(t1[:], HR[ot][:].bitcast(F32), muf[:])
                    nc.vector.tensor_mul(t1[:], t1[:], rstdf[:])
                    nc.vector.tensor_scalar(H1[ot][:], t1[:], bcol("g1", ot),
                                            bcol("be1", ot), ALU.mult, ALU.add)
                    nc.vector.tensor_copy(H1b[ot][:], H1[ot][:].bitcast(F32))
            s_big.close()   # free KT/VA/QT/ATT/tables/xq before FFN tiles

            # ============ Phase F: FFN + residual + LN2 ============
            with ExitStack() as ctx:
                ffp = ctx.enter_context(tc.tile_pool(name="pFF", bufs=1))
                scr = ctx.enter_context(tc.tile_pool(name="scrF", bufs=1))
                stat = ctx.enter_context(tc.tile_pool(name="statF", bufs=1))
                grp = ctx.enter_context(tc.tile_pool(name="grp", bufs=1))
                psStat = ctx.enter_context(tc.tile_pool(name="psStatF", bufs=1,
                                                        space="PSUM"))
                pstat2 = psStat.tile([1, 1024], F32, tag="pstat2")
                pSum2, pSq2 = pstat2[:, 0:512], pstat2[:, 512:1024]
                FFT = [ffp.tile([128, Q], BF16, tag=f"ff{i}", name=f"FFT{i}")
                       for i in range(NT_F)]
                w1v = w1P.ap().rearrange("p (kt c) -> p kt c", c=F)
                with tc.tile_pool(name="w1p", bufs=1) as w1p, \
                        tc.tile_pool(name="psF", bufs=1, space="PSUM") as psF:
                    for fb in range(F // 512):
                        w1b = w1p.tile([128, NT_D * 512], BF16, tag="w1",
                                       bufs=4, name=f"w1b{fb}")
                        nc.sync.dma_start(
                            w1b[:].rearrange("p (kt c) -> p kt c", c=512),
                            w1v[:, :, fb * 512:(fb + 1) * 512])
                        for j in range(4):
                            ft = fb * 4 + j
                            pf = psF.tile([128, Q], F32, tag="pf", bufs=2,
                                          name=f"pf{ft}")
                            for kt in range(NT_D):
                                nc.tensor.matmul(
                                    pf[:],
                                    lhsT=w1b[:, kt * 512 + j * 128:
                                             kt * 512 + (j + 1) * 128],
                                    rhs=H1b[kt][:],
                                    start=(kt == 0), stop=(kt == NT_D - 1))
                            nc.scalar.activation(FFT[ft][:], pf[:], AF.Relu,
                                                 bias=bcol("b1", ft))
                GR = [grp.tile([128, Q], F32R, tag=f"gr{i}", name=f"GR{i}")
                      for i in range(NT_D)]
                w2v = w2P.ap().rearrange("p (ft c) -> p ft c", c=D)
                with tc.tile_pool(name="w2p", bufs=1) as w2p, \
                        tc.tile_pool(name="psG", bufs=1, space="PSUM") as psG:
                    for ot in range(NT_D):
                        w2b = w2p.tile([128, NT_F * 128], BF16, tag="w2",
                                       bufs=3, name=f"w2b{ot}")
                        # w2 on the ACT ring: on SP it queues behind w1's
                        # slot-WAR-blocked preps (head-of-line) and stalls
                        # the w1->w2 transition
                        nc.scalar.dma_start(
                            w2b[:].rearrange("p (ft c) -> p ft c", c=128),
                            w2v[:, :, ot * 128:(ot + 1) * 128])
                        pg = psG.tile([128, Q], F32, tag="pg", bufs=2,
                                      name=f"pg{ot}")
                        for ft in range(NT_F):
                            nc.tensor.matmul(
                                pg[:], lhsT=w2b[:, ft * 128:(ft + 1) * 128],
                                rhs=FFT[ft][:],
                                start=(ft == 0), stop=(ft == NT_F - 1))
                        go = scr.tile([128, Q], F32, tag="go", bufs=3,
                                      name=f"goF{ot}")
                        nc.vector.tensor_scalar_add(go[:], pg[:],
                                                    bcol("b2", ot))
                        nc.vector.tensor_add(GR[ot][:], go[:],
                                             H1[ot][:].bitcast(F32))
                        sq2 = scr.tile([128, Q], F32R, tag="sq2", bufs=2,
                                       name=f"sq2F{ot}")
                        nc.vector.tensor_mul(sq2[:], GR[ot][:].bitcast(F32),
                                             GR[ot][:].bitcast(F32))
                        nc.tensor.matmul(pSum2, lhsT=ones_c, rhs=GR[ot][:],
                                         start=(ot == 0), stop=(ot == NT_D - 1))
                        nc.tensor.matmul(pSq2, lhsT=ones_c, rhs=sq2[:],
                                         start=(ot == 0), stop=(ot == NT_D - 1))
                mu = stat.tile([1, Q], F32, tag="mu")
                nc.vector.tensor_scalar_mul(mu[:], pSum2, 1.0 / D)
                var = stat.tile([1, Q], F32, tag="var")
                nc.vector.tensor_scalar_mul(var[:], pSq2, 1.0 / D)
                mu2 = stat.tile([1, Q], F32, tag="mu2")
                nc.vector.tensor_mul(mu2[:], mu[:], mu[:])
                nc.vector.tensor_sub(var[:], var[:], mu2[:])
                nc.vector.tensor_scalar_add(var[:], var[:], LN_EPS)
                lnv = stat.tile([1, Q], F32, tag="lnv")
                nc.scalar.activation(lnv[:], var[:], AF.Ln)
                rstd = stat.tile([1, Q], F32, tag="rstd")
                nc.scalar.activation(rstd[:], lnv[:], AF.Exp, scale=-0.5)
                muf = stat.tile([128, Q], F32, tag="muf")
                nc.gpsimd.partition_broadcast(muf[:], mu[:], channels=128)
                rstdf = stat.tile([128, Q], F32, tag="rstdf")
                nc.gpsimd.partition_broadcast(rstdf[:], rstd[:], channels=128)
                for ot in range(NT_D):
                    t1 = scr.tile([128, Q], F32, tag="t1f", bufs=2,
                                  name=f"t1F{ot}")
                    nc.vector.tensor_sub(t1[:], GR[ot][:].bitcast(F32), muf[:])
                    nc.vector.tensor_mul(t1[:], t1[:], rstdf[:])
                    yt = scr.tile([128, Q], F32, tag="yt", bufs=2,
                                  name=f"ytF{ot}")
                    nc.vector.tensor_scalar(yt[:], t1[:], bcol("g2", ot),
                                            bcol("be2", ot), ALU.mult, ALU.add)
                    # stores ride the SP ring: on the ACT ring their preps
                    # (gated by the serial LN2 DVE chain) head-of-line block
                    # the NEXT rep's x/wq/wk loads
                    nc.sync.dma_start(yT.ap()[ot * 128:(ot + 1) * 128, :],
                                      yt[:])
            s_EF.close()

    nc.compile()
    return nc


def _rope_tables():
    inv_freq = (1.0 / (ROPE_BASE ** (np.arange(0, Dh, 2, dtype=np.float32) / Dh)))
    angles = np.arange(S, dtype=np.float32)[:, None] * inv_freq[None, :]
    cos = np.cos(angles).T.astype(np.float32)   # (32, S)
    sin = np.sin(angles).T.astype(np.float32)
    cosK = np.concatenate([cos, cos, cos, cos], axis=0)          # (128, S)
    sinDK = np.concatenate([-sin, sin, -sin, sin], axis=0)
    return np.ascontiguousarray(cosK), np.ascontiguousarray(sinDK)


def _pack_w(wT):
    """(D, D) W^T -> (128, 8192) bf16: [p, kt*1024 + c] = wT[kt*128+p, c]."""
    w = np.asarray(wT, dtype=np.float32).reshape(NT_D, 128, D)
    return np.ascontiguousarray(
        w.transpose(1, 0, 2).reshape(128, NT_D * D).astype(ml_dtypes.bfloat16))


def _in_maps(x, in_proj_w, in_proj_b, out_w, out_b, w1, b1, w2, b2,
             ln1_g, ln1_b, ln2_g, ln2_b):
    x = np.asarray(x, dtype=np.float32)
    f32 = lambda a: np.ascontiguousarray(np.asarray(a, dtype=np.float32))
    bf16 = lambda a: np.ascontiguousarray(
        np.asarray(a, dtype=np.float32).astype(ml_dtypes.bfloat16))

    perm = np.concatenate(
        [h * Dh + np.concatenate([np.arange(0, Dh, 2), np.arange(1, Dh, 2)])
         for h in range(H)])
    wq = np.asarray(in_proj_w)[0:D][perm]
    wk = np.asarray(in_proj_w)[D:2 * D][perm]
    wv = np.asarray(in_proj_w)[2 * D:3 * D]
    bqv = np.asarray(in_proj_b)[0:D][perm]
    bkv = np.asarray(in_proj_b)[D:2 * D][perm]
    bvv = np.asarray(in_proj_b)[2 * D:3 * D]
    cosK, sinDK = _rope_tables()

    w1T = np.asarray(w1, dtype=np.float32).T          # (D, F)
    w2T = np.asarray(w2, dtype=np.float32).T          # (F, D)
    w1Pm = np.ascontiguousarray(
        w1T.reshape(NT_D, 128, F).transpose(1, 0, 2).reshape(128, NT_D * F)
        .astype(ml_dtypes.bfloat16))
    w2Pm = np.ascontiguousarray(
        w2T.reshape(NT_F, 128, D).transpose(1, 0, 2).reshape(128, NT_F * D)
        .astype(ml_dtypes.bfloat16))

    bpack = np.zeros((128, BP_COLS), np.float32)

    def put(key, vec):
        v = np.asarray(vec, dtype=np.float32).reshape(-1)
        n = v.size // 128
        bpack[:, _BP[key]:_BP[key] + n] = v.reshape(n, 128).T
    ob_eff = (np.asarray(out_b, dtype=np.float64) +
              np.asarray(out_w, dtype=np.float64) @
              np.asarray(bvv, dtype=np.float64)).astype(np.float32)
    put("bq", bqv); put("bk", bkv); put("ob", ob_eff); put("b2", b2)
    put("g1", ln1_g); put("be1", ln1_b); put("g2", ln2_g); put("be2", ln2_b)
    put("b1", b1)
    bpack[:, _BP["ones"]:_BP["ones"] + 16] = 1.0

    shared = {
        "wqP": _pack_w(wq.T), "wkP": _pack_w(wk.T), "wvP": _pack_w(wv.T),
        "owP": _pack_w(np.asarray(out_w).T),
        "w1P": w1Pm, "w2P": w2Pm,
        "bpack": bpack,
    }
    in_maps = []
    for c in range(8):
        b_, qb = c // 4, c % 4
        q0 = qb * Q
        xT = x[b_].T                                   # (D, S)
        # packed x: chunk order [qb, others...]; chunk i block kt at
        # cols i*4096 + kt*512; K rope tables permuted to match
        order = [qb] + [s for s in range(NC_S) if s != qb]
        xPm = np.empty((128, NC_S * 4096), np.float32)
        cosKP = np.empty((128, S), np.float32)
        sinKP = np.empty((128, S), np.float32)
        for i, sc in enumerate(order):
            blk = xT[:, sc * 512:(sc + 1) * 512]       # (D, 512)
            xPm[:, i * 4096:(i + 1) * 4096] = (
                blk.reshape(NT_D, 128, 512).transpose(1, 0, 2)
                .reshape(128, 4096))
            cosKP[:, i * 512:(i + 1) * 512] = cosK[:, sc * 512:(sc + 1) * 512]
            sinKP[:, i * 512:(i + 1) * 512] = sinDK[:, sc * 512:(sc + 1) * 512]
        m = dict(shared)
        m["xPb"] = np.ascontiguousarray(xPm.astype(ml_dtypes.bfloat16))
        m["xQf"] = np.ascontiguousarray(xPm[:, 0:4096])
        m["cosKb"] = np.ascontiguousarray(cosKP.astype(ml_dtypes.bfloat16))
        m["sinDKb"] = np.ascontiguousarray(sinKP.astype(ml_dtypes.bfloat16))
        in_maps.append(m)
    return in_maps


def kernel(x, in_proj_w, in_proj_b, out_w, out_b, w1, b1, w2, b2,
           ln1_g, ln1_b, ln2_g, ln2_b):
    if "nc" not in _CACHE:
        _CACHE["nc"] = _build()
    nc = _CACHE["nc"]
    in_maps = _in_maps(x, in_proj_w, in_proj_b, out_w, out_b, w1, b1, w2, b2,
                       ln1_g, ln1_b, ln2_g, ln2_b)
    res = run_bass_kernel_spmd(nc, in_maps, core_ids=list(range(8)))
    out = np.empty((B, S, D), dtype=np.float32)
    for c in range(8):
        b_, qb = c // 4, c % 4
        out[b_, qb * Q:(qb + 1) * Q, :] = res.results[c]["yT"].T
    return out
